# revision 17
# baseline (speedup 1.0000x reference)
"""Trainium2 Bass kernel for the AGA retrieval-KNN operator (8-core SPMD).

Reference computation (per token):
    q = hs @ Wq.T                        [BD]
    s = q @ K.T / sqrt(BD) + log(rel)    [N]
    top8, idx = top_k(s, 8); attn = softmax(top8); gate = sigmoid(top8[0])
    aux = attn @ V[idx]                  [H]
    aux = gelu(aux @ Wd.T) @ Wu.T        [H]
    out = pao + gate * aux

Kernel strategy:
  - Pure data parallel: 8192 tokens sharded 1024/core; weights replicated.
  - Host-side constant folding: WsT = (K @ Wq).T / sqrt(BD)  [H, N] so the
    score is a single matmul; Vd = V @ Wd.T [N, VBD] so the top-k gather +
    weighted sum + down-projection collapse into one dense [T,N]x[N,VBD]
    matmul against a sparse (top-8 masked) softmax weight vector;
    WuT = 0.5 * Wu.T (0.5 from the tanh-gelu identity gelu(x)=0.5x(1+t)).
  - Top-8 via the DVE max8 instruction; dense masked softmax weights via one
    scalar_tensor_tensor (mask * exp, accumulating the denominator).
  - 1/denom and the sigmoid gate are folded into per-partition scalar
    operands of downstream ops (they commute with the linear ops).
  - gelu computed with the tanh formula (exp/tanh share one ACT table set;
    the Gelu LUT lives in a different set and would thrash table loads).
  - pao is accumulated into the up-projection PSUM with an identity-matmul.
  - bf16 storage/compute everywhere (fp32 PSUM accumulate); fp32 host cast.
"""

import numpy as np
import ml_dtypes

B, S, H = 4, 2048, 2048
NSLOT = 256  # slot pool size == BD == VBD
P = 128
N_CORES = 8
TPC = (B * S) // N_CORES  # tokens per core = 1024
NT = TPC // P  # token tiles per core = 8
KH = H // P  # contraction chunks over H = 16
BF16 = ml_dtypes.bfloat16

_CACHE = {}


def _build_graph():
    import concourse.bacc as bacc
    import concourse.mybir as mybir
    from concourse import tile

    F32 = mybir.dt.float32
    BF = mybir.dt.bfloat16
    AF = mybir.ActivationFunctionType
    ALU = mybir.AluOpType

    nc = bacc.Bacc("TRN2", target_bir_lowering=False, debug=False)

    NP = NSLOT
    hst = nc.dram_tensor("hst", [H, TPC], BF, kind="ExternalInput")
    paob = nc.dram_tensor("paob", [TPC, H], BF, kind="ExternalInput")
    wst = nc.dram_tensor("wst", [H, NP], BF, kind="ExternalInput")
    vd = nc.dram_tensor("vd", [NSLOT, NSLOT], BF, kind="ExternalInput")
    wut = nc.dram_tensor("wut", [NSLOT, H], BF, kind="ExternalInput")
    logrel = nc.dram_tensor("logrel", [P, NP], F32, kind="ExternalInput")
    ident = nc.dram_tensor("ident", [P, P], BF, kind="ExternalInput")
    out = nc.dram_tensor("out", [TPC, H], BF, kind="ExternalOutput")

    with tile.TileContext(nc) as tc:
        with (
            tc.tile_pool(name="const", bufs=1) as cpool,
            tc.tile_pool(name="work", bufs=4) as wpool,
            tc.tile_pool(name="io", bufs=3) as iopool,
            tc.tile_pool(name="ps_s", bufs=2, space="PSUM") as ps_s_pool,
            tc.tile_pool(name="ps_tr", bufs=2, space="PSUM") as ps_tr_pool,
            tc.tile_pool(name="ps_g", bufs=1, space="PSUM") as ps_g_pool,
            tc.tile_pool(name="ps_u", bufs=3, space="PSUM") as ps_u_pool,
        ):
            # ---- resident weights/activations -------------------------------
            # hsT/WsT loaded per 128-row chunk so the k-th score matmul can
            # start as soon as its chunk lands.
            hsT = cpool.tile([P, KH, TPC], BF)
            wst_sb = cpool.tile([P, KH, NP], BF)
            for k in range(KH):
                nc.sync.dma_start(
                    out=wst_sb[:, k, :], in_=wst.ap()[k * P : (k + 1) * P, :]
                )
                nc.sync.dma_start(
                    out=hsT[:, k, :], in_=hst.ap()[k * P : (k + 1) * P, :]
                )
            vd_sb = cpool.tile([P, 2, NSLOT], BF)
            nc.sync.dma_start(
                out=vd_sb[:], in_=vd.ap().rearrange("(k p) n -> p k n", p=P)
            )
            wut_sb = cpool.tile([P, 2, H], BF)
            nc.sync.dma_start(
                out=wut_sb[:], in_=wut.ap().rearrange("(k p) n -> p k n", p=P)
            )
            logrel_sb = cpool.tile([P, NP], F32)
            nc.sync.dma_start(out=logrel_sb[:], in_=logrel.ap())
            ident_sb = cpool.tile([P, P], BF)
            nc.sync.dma_start(out=ident_sb[:], in_=ident.ap())

            for t in range(NT):
                tok = slice(t * P, (t + 1) * P)

                # ---- scores: s = hs @ WsT + logrel --------------------------
                ps_s = ps_s_pool.tile([P, NP], F32)
                for k in range(KH):
                    nc.tensor.matmul(
                        out=ps_s[:],
                        lhsT=hsT[:, k, tok],
                        rhs=wst_sb[:, k, :],
                        start=(k == 0),
                        stop=(k == KH - 1),
                    )
                s_sb = wpool.tile([P, NP], F32, tag="s")
                nc.vector.tensor_tensor(
                    out=s_sb[:], in0=ps_s[:], in1=logrel_sb[:], op=ALU.add
                )

                # ---- top-8, gate, masked softmax weights --------------------
                t8 = wpool.tile([P, 8], F32, tag="t8")
                nc.vector.max(out=t8[:], in_=s_sb[:])
                neg_m = wpool.tile([P, 1], F32, tag="negm")
                nc.vector.tensor_scalar_mul(neg_m[:], t8[:, 0:1], -1.0)
                e_sb = wpool.tile([P, NP], BF, tag="e")
                nc.scalar.activation(e_sb[:], s_sb[:], AF.Exp, bias=neg_m[:])
                pair = wpool.tile([P, 2], F32, tag="pair")
                # pair[:,1] = exp(-top1)
                nc.scalar.activation(pair[:, 1:2], t8[:, 0:1], AF.Exp, scale=-1.0)
                nc.vector.tensor_scalar_add(pair[:, 1:2], pair[:, 1:2], 1.0)
                w_sb = wpool.tile([P, NSLOT], BF, tag="w")
                # w = (s >= top8) * exp(s - top1); pair[:,0] = sum(w) = denom
                nc.vector.scalar_tensor_tensor(
                    out=w_sb[:],
                    in0=s_sb[:],
                    scalar=t8[:, 7:8],
                    in1=e_sb[:],
                    op0=ALU.is_ge,
                    op1=ALU.mult,
                    accum_out=pair[:, 0:1],
                )
                rec = wpool.tile([P, 2], F32, tag="rec")
                # rec[:,0] = 1/denom ; rec[:,1] = gate = sigmoid(top1)
                nc.vector.reciprocal(rec[:], pair[:])

                # ---- transpose w, g = w @ Vd --------------------------------
                wT = wpool.tile([P, 2, P], BF, tag="wT")
                for k in range(2):
                    ps_tr = ps_tr_pool.tile([P, P], BF, tag="ptr")
                    nc.tensor.transpose(
                        ps_tr[:], w_sb[:, k * P : (k + 1) * P], ident_sb[:]
                    )
                    nc.vector.tensor_copy(wT[:, k, :], ps_tr[:])
                ps_g = ps_g_pool.tile([P, NSLOT], F32)
                for k in range(2):
                    nc.tensor.matmul(
                        out=ps_g[:],
                        lhsT=wT[:, k, :],
                        rhs=vd_sb[:, k, :],
                        start=(k == 0),
                        stop=(k == 1),
                    )

                # ---- tanh-gelu with 1/denom and gate folded in --------------
                # x = g/denom tiny (|x| < 0.3) so gelu(x) ~ 0.5 x (1+tanh(c0 x))
                # (x^3 term < 1e-5 relative; 0.5 folded into WuT). Both reads
                # come straight from PSUM with fused per-partition scales, in
                # parallel on DVE (xg) and ACT (tanh):
                #   xg = g * (gate/denom) ; r = tanh(g * (c0/denom))
                #   d = (r + 1) * xg
                sc = wpool.tile([P, 2], F32, tag="sc")
                nc.vector.tensor_tensor(
                    out=sc[:, 0:1], in0=rec[:, 0:1], in1=rec[:, 1:2], op=ALU.mult
                )
                nc.vector.tensor_scalar_mul(sc[:, 1:2], rec[:, 0:1], 0.7978845608028654)
                pao_t = iopool.tile([P, H], BF, tag="pao")
                nc.sync.dma_start(out=pao_t[:], in_=paob.ap()[tok, :])
                rr = wpool.tile([P, NSLOT], BF, tag="rr")
                nc.scalar.activation(rr[:], ps_g[:], AF.Tanh, scale=sc[:, 1:2])
                x_sb = wpool.tile([P, NSLOT], BF, tag="x")
                nc.vector.tensor_scalar_mul(x_sb[:], ps_g[:], sc[:, 0:1])
                d_sb = wpool.tile([P, NSLOT], BF, tag="d")
                nc.vector.scalar_tensor_tensor(
                    out=d_sb[:], in0=rr[:], scalar=1.0, in1=x_sb[:],
                    op0=ALU.add, op1=ALU.mult,
                )

                # ---- transpose d, u = d @ WuT + pao -------------------------
                dT = wpool.tile([P, 2, P], BF, tag="dT")
                for k in range(2):
                    ps_tr = ps_tr_pool.tile([P, P], BF, tag="ptr")
                    nc.tensor.transpose(
                        ps_tr[:], d_sb[:, k * P : (k + 1) * P], ident_sb[:]
                    )
                    nc.scalar.activation(dT[:, k, :], ps_tr[:], AF.Copy)

                out_sb = iopool.tile([P, H], BF, tag="out")
                for c in range(4):
                    ps_u = ps_u_pool.tile([P, 512], F32, tag="pu", name=f"pu{t}{c}")
                    cs = slice(c * 512, (c + 1) * 512)
                    nc.tensor.matmul(
                        out=ps_u[:], lhsT=dT[:, 0, :], rhs=wut_sb[:, 0, cs],
                        start=True, stop=False,
                    )
                    last_pe = c < 2
                    nc.tensor.matmul(
                        out=ps_u[:], lhsT=dT[:, 1, :], rhs=wut_sb[:, 1, cs],
                        start=False, stop=not last_pe,
                    )
                    if last_pe:
                        # pao folded in on the PE; evacuate on ACT
                        nc.tensor.matmul(
                            out=ps_u[:], lhsT=ident_sb[:], rhs=pao_t[:, cs],
                            start=False, stop=True,
                        )
                        nc.scalar.activation(out_sb[:, cs], ps_u[:], AF.Copy)
                    else:
                        # pao folded into the DVE evacuation
                        nc.vector.scalar_tensor_tensor(
                            out=out_sb[:, cs], in0=ps_u[:], scalar=0.0, in1=pao_t[:, cs],
                            op0=ALU.add, op1=ALU.add,
                        )
                nc.sync.dma_start(out=out.ap()[tok, :], in_=out_sb[:])

    nc.compile()
    return nc


def _get_graph():
    if "nc" not in _CACHE:
        _CACHE["nc"] = _build_graph()
    return _CACHE["nc"]


def _make_in_maps(
    hidden_states,
    primary_attention_output,
    q_proj_w,
    slot_keys,
    slot_values,
    reliability,
    value_down_w,
    value_up_w,
):
    hs2 = np.asarray(hidden_states, np.float32).reshape(-1, H)
    pao2 = np.asarray(primary_attention_output, np.float32).reshape(-1, H)
    wq = np.asarray(q_proj_w, np.float32)
    kk = np.asarray(slot_keys, np.float32)
    vv = np.asarray(slot_values, np.float32)
    rel = np.asarray(reliability, np.float32)
    wd = np.asarray(value_down_w, np.float32)
    wu = np.asarray(value_up_w, np.float32)

    bd = wq.shape[0]
    wst_h = ((kk @ wq) / np.sqrt(np.float32(bd))).T  # [H, N]
    vd_h = vv @ wd.T  # [N, VBD]
    wut_h = 0.5 * wu.T  # [VBD, H]
    logrel_h = np.broadcast_to(
        np.log(np.clip(rel, 1e-10, None)), (P, NSLOT)
    ).astype(np.float32).copy()
    ident_h = np.eye(P, dtype=np.float32)

    shared = {
        "wst": np.ascontiguousarray(wst_h).astype(BF16),
        "vd": np.ascontiguousarray(vd_h).astype(BF16),
        "wut": np.ascontiguousarray(wut_h).astype(BF16),
        "logrel": logrel_h,
        "ident": ident_h.astype(BF16),
    }
    hs2b = hs2.astype(BF16)
    in_maps = []
    for c in range(N_CORES):
        rows = slice(c * TPC, (c + 1) * TPC)
        in_maps.append(
            {
                "hst": np.ascontiguousarray(hs2b[rows].T),
                "paob": np.ascontiguousarray(pao2[rows]).astype(BF16),
                **shared,
            }
        )
    return in_maps


def kernel(**inputs):
    from concourse.bass_utils import run_bass_kernel_spmd

    nc = _get_graph()
    in_maps = _make_in_maps(**inputs)
    res = run_bass_kernel_spmd(nc, in_maps, core_ids=list(range(N_CORES)))
    full = np.concatenate(
        [res.results[c]["out"].astype(np.float32) for c in range(N_CORES)], axis=0
    )
    return full.reshape(B, S, H)


# revision 19
# speedup vs baseline: 1.0047x; 1.0047x over previous
"""Trainium2 Bass kernel for the AGA retrieval-KNN operator (8-core SPMD).

Reference computation (per token):
    q = hs @ Wq.T                        [BD]
    s = q @ K.T / sqrt(BD) + log(rel)    [N]
    top8, idx = top_k(s, 8); attn = softmax(top8); gate = sigmoid(top8[0])
    aux = attn @ V[idx]                  [H]
    aux = gelu(aux @ Wd.T) @ Wu.T        [H]
    out = pao + gate * aux

Kernel strategy:
  - Pure data parallel: 8192 tokens sharded 1024/core; weights replicated.
  - Host-side constant folding: WsT = (K @ Wq).T / sqrt(BD)  [H, N] so the
    score is a single matmul; Vd = V @ Wd.T [N, VBD] so the top-k gather +
    weighted sum + down-projection collapse into one dense [T,N]x[N,VBD]
    matmul against a sparse (top-8 masked) softmax weight vector;
    WuT = 0.5 * Wu.T (0.5 from the tanh-gelu identity gelu(x)=0.5x(1+t)).
  - Top-8 via the DVE max8 instruction; dense masked softmax weights via one
    scalar_tensor_tensor (mask * exp, accumulating the denominator).
  - 1/denom and the sigmoid gate are folded into per-partition scalar
    operands of downstream ops (they commute with the linear ops).
  - gelu computed with the tanh formula (exp/tanh share one ACT table set;
    the Gelu LUT lives in a different set and would thrash table loads).
  - pao is accumulated into the up-projection PSUM with an identity-matmul.
  - bf16 storage/compute everywhere (fp32 PSUM accumulate); fp32 host cast.
"""

import numpy as np
import ml_dtypes

B, S, H = 4, 2048, 2048
NSLOT = 256  # slot pool size == BD == VBD
P = 128
N_CORES = 8
TPC = (B * S) // N_CORES  # tokens per core = 1024
NT = TPC // P  # token tiles per core = 8
KH = H // P  # contraction chunks over H = 16
BF16 = ml_dtypes.bfloat16

_CACHE = {}


def _build_graph():
    import concourse.bacc as bacc
    import concourse.mybir as mybir
    from concourse import tile

    F32 = mybir.dt.float32
    BF = mybir.dt.bfloat16
    AF = mybir.ActivationFunctionType
    ALU = mybir.AluOpType

    nc = bacc.Bacc("TRN2", target_bir_lowering=False, debug=False)

    NP = NSLOT
    hst = nc.dram_tensor("hst", [H, TPC], BF, kind="ExternalInput")
    paob = nc.dram_tensor("paob", [TPC, H], BF, kind="ExternalInput")
    wst = nc.dram_tensor("wst", [H, NP], BF, kind="ExternalInput")
    vd = nc.dram_tensor("vd", [NSLOT, NSLOT], BF, kind="ExternalInput")
    wut = nc.dram_tensor("wut", [NSLOT, H], BF, kind="ExternalInput")
    logrel = nc.dram_tensor("logrel", [P, NP], F32, kind="ExternalInput")
    ident = nc.dram_tensor("ident", [P, P], BF, kind="ExternalInput")
    out = nc.dram_tensor("out", [TPC, H], BF, kind="ExternalOutput")

    with tile.TileContext(nc) as tc:
        with (
            tc.tile_pool(name="const", bufs=1) as cpool,
            tc.tile_pool(name="work", bufs=4) as wpool,
            tc.tile_pool(name="io", bufs=3) as iopool,
            tc.tile_pool(name="ps_s", bufs=2, space="PSUM") as ps_s_pool,
            tc.tile_pool(name="ps_tr", bufs=2, space="PSUM") as ps_tr_pool,
            tc.tile_pool(name="ps_g", bufs=1, space="PSUM") as ps_g_pool,
            tc.tile_pool(name="ps_u", bufs=3, space="PSUM") as ps_u_pool,
        ):
            # ---- resident weights/activations -------------------------------
            # hsT/WsT loaded per 128-row chunk so the k-th score matmul can
            # start as soon as its chunk lands.
            hsT = cpool.tile([P, KH, TPC], BF)
            wst_sb = cpool.tile([P, KH, NP], BF)
            for k in range(KH):
                nc.sync.dma_start(
                    out=wst_sb[:, k, :], in_=wst.ap()[k * P : (k + 1) * P, :]
                )
                nc.sync.dma_start(
                    out=hsT[:, k, :], in_=hst.ap()[k * P : (k + 1) * P, :]
                )
            vd_sb = cpool.tile([P, 2, NSLOT], BF)
            nc.sync.dma_start(
                out=vd_sb[:], in_=vd.ap().rearrange("(k p) n -> p k n", p=P)
            )
            wut_sb = cpool.tile([P, 2, H], BF)
            nc.sync.dma_start(
                out=wut_sb[:], in_=wut.ap().rearrange("(k p) n -> p k n", p=P)
            )
            logrel_sb = cpool.tile([P, NP], F32)
            nc.sync.dma_start(out=logrel_sb[:], in_=logrel.ap())
            ident_sb = cpool.tile([P, P], BF)
            nc.sync.dma_start(out=ident_sb[:], in_=ident.ap())

            state = {}

            def phase_a(t):
                """scores -> top8 -> masked softmax weights w, scales."""
                tok = slice(t * P, (t + 1) * P)
                ps_s = ps_s_pool.tile([P, NP], F32, tag="ps_s", name=f"ps_s{t}")
                for k in range(KH):
                    nc.tensor.matmul(
                        out=ps_s[:],
                        lhsT=hsT[:, k, tok],
                        rhs=wst_sb[:, k, :],
                        start=(k == 0),
                        stop=(k == KH - 1),
                    )
                s_sb = wpool.tile([P, NP], F32, tag="s", name=f"s{t}")
                nc.vector.tensor_tensor(
                    out=s_sb[:], in0=ps_s[:], in1=logrel_sb[:], op=ALU.add
                )
                t8 = wpool.tile([P, 8], F32, tag="t8", name=f"t8_{t}")
                nc.vector.max(out=t8[:], in_=s_sb[:])
                neg_m = wpool.tile([P, 1], F32, tag="negm", name=f"negm{t}")
                nc.vector.tensor_scalar_mul(neg_m[:], t8[:, 0:1], -1.0)
                e_sb = wpool.tile([P, NP], BF, tag="e", name=f"e{t}")
                nc.scalar.activation(e_sb[:], s_sb[:], AF.Exp, bias=neg_m[:])
                pair = wpool.tile([P, 2], F32, tag="pair", name=f"pair{t}")
                # pair[:,1] = exp(-top1)
                nc.scalar.activation(pair[:, 1:2], t8[:, 0:1], AF.Exp, scale=-1.0)
                nc.vector.tensor_scalar_add(pair[:, 1:2], pair[:, 1:2], 1.0)
                w_sb = wpool.tile([P, NSLOT], BF, tag="w", name=f"w{t}")
                # w = (s >= top8) * exp(s - top1); pair[:,0] = sum(w) = denom
                nc.vector.scalar_tensor_tensor(
                    out=w_sb[:],
                    in0=s_sb[:],
                    scalar=t8[:, 7:8],
                    in1=e_sb[:],
                    op0=ALU.is_ge,
                    op1=ALU.mult,
                    accum_out=pair[:, 0:1],
                )
                rec = wpool.tile([P, 2], F32, tag="rec", name=f"rec{t}")
                # rec[:,0] = 1/denom ; rec[:,1] = gate = sigmoid(top1)
                nc.vector.reciprocal(rec[:], pair[:])
                sc = wpool.tile([P, 2], F32, tag="sc", name=f"sc{t}")
                # sc[:,0] = gate/denom ; sc[:,1] = c0/denom (tanh-gelu const)
                nc.vector.tensor_tensor(
                    out=sc[:, 0:1], in0=rec[:, 0:1], in1=rec[:, 1:2], op=ALU.mult
                )
                nc.vector.tensor_scalar_mul(sc[:, 1:2], rec[:, 0:1], 0.7978845608028654)
                pao_t = iopool.tile([P, H], BF, tag="pao", name=f"pao{t}")
                nc.sync.dma_start(out=pao_t[:], in_=paob.ap()[tok, :])
                state[t] = (w_sb, sc, pao_t)

            def phase_b(t):
                """w -> g -> gelu -> u -> + pao -> out."""
                tok = slice(t * P, (t + 1) * P)
                w_sb, sc, pao_t = state.pop(t)
                wT = wpool.tile([P, 2, P], BF, tag="wT", name=f"wT{t}")
                for k in range(2):
                    ps_tr = ps_tr_pool.tile([P, P], BF, tag="ptr", name=f"ptw{t}{k}")
                    nc.tensor.transpose(
                        ps_tr[:], w_sb[:, k * P : (k + 1) * P], ident_sb[:]
                    )
                    nc.vector.tensor_copy(wT[:, k, :], ps_tr[:])
                ps_g = ps_g_pool.tile([P, NSLOT], F32, tag="ps_g", name=f"ps_g{t}")
                for k in range(2):
                    nc.tensor.matmul(
                        out=ps_g[:],
                        lhsT=wT[:, k, :],
                        rhs=vd_sb[:, k, :],
                        start=(k == 0),
                        stop=(k == 1),
                    )
                # gelu(x) ~ 0.5 x (1 + tanh(c0 x)) for tiny x = g/denom (the
                # x^3 term is < 1e-5 relative here; 0.5 folded into WuT).
                # d = (1 + tanh(g * c0/denom)) * g * (gate/denom)
                rr = wpool.tile([P, NSLOT], BF, tag="rr", name=f"rr{t}")
                nc.scalar.activation(rr[:], ps_g[:], AF.Tanh, scale=sc[:, 1:2])
                dp = wpool.tile([P, NSLOT], BF, tag="dp", name=f"dp{t}")
                nc.vector.scalar_tensor_tensor(
                    out=dp[:], in0=rr[:], scalar=1.0, in1=ps_g[:],
                    op0=ALU.add, op1=ALU.mult,
                )
                d_sb = wpool.tile([P, NSLOT], BF, tag="d", name=f"d{t}")
                nc.vector.tensor_scalar_mul(d_sb[:], dp[:], sc[:, 0:1])

                dT = wpool.tile([P, 2, P], BF, tag="dT", name=f"dT{t}")
                for k in range(2):
                    ps_tr = ps_tr_pool.tile([P, P], BF, tag="ptr", name=f"ptd{t}{k}")
                    nc.tensor.transpose(
                        ps_tr[:], d_sb[:, k * P : (k + 1) * P], ident_sb[:]
                    )
                    nc.scalar.activation(dT[:, k, :], ps_tr[:], AF.Copy)

                out_sb = iopool.tile([P, H], BF, tag="out", name=f"o{t}")
                for c in range(4):
                    ps_u = ps_u_pool.tile([P, 512], F32, tag="pu", name=f"pu{t}{c}")
                    cs = slice(c * 512, (c + 1) * 512)
                    nc.tensor.matmul(
                        out=ps_u[:], lhsT=dT[:, 0, :], rhs=wut_sb[:, 0, cs],
                        start=True, stop=False,
                    )
                    with_pe_pao = c < 2
                    nc.tensor.matmul(
                        out=ps_u[:], lhsT=dT[:, 1, :], rhs=wut_sb[:, 1, cs],
                        start=False, stop=not with_pe_pao,
                    )
                    if with_pe_pao:
                        # pao folded in on the PE; evacuate on ACT
                        nc.tensor.matmul(
                            out=ps_u[:], lhsT=ident_sb[:], rhs=pao_t[:, cs],
                            start=False, stop=True,
                        )
                        nc.scalar.activation(out_sb[:, cs], ps_u[:], AF.Copy)
                    else:
                        # pao folded into the DVE evacuation
                        nc.vector.scalar_tensor_tensor(
                            out=out_sb[:, cs], in0=ps_u[:], scalar=0.0,
                            in1=pao_t[:, cs], op0=ALU.add, op1=ALU.add,
                        )
                nc.sync.dma_start(out=out.ap()[tok, :], in_=out_sb[:])

            # software pipeline: emit phase-A of tile t+1 before phase-B of
            # tile t so each engine's (in-order) stream interleaves the two --
            # PE fills tile t's vector/scalar latency with tile t+1's scores.
            phase_a(0)
            for t in range(NT):
                if t + 1 < NT:
                    phase_a(t + 1)
                phase_b(t)

    nc.compile()
    return nc


def _get_graph():
    if "nc" not in _CACHE:
        _CACHE["nc"] = _build_graph()
    return _CACHE["nc"]


def _make_in_maps(
    hidden_states,
    primary_attention_output,
    q_proj_w,
    slot_keys,
    slot_values,
    reliability,
    value_down_w,
    value_up_w,
):
    hs2 = np.asarray(hidden_states, np.float32).reshape(-1, H)
    pao2 = np.asarray(primary_attention_output, np.float32).reshape(-1, H)
    wq = np.asarray(q_proj_w, np.float32)
    kk = np.asarray(slot_keys, np.float32)
    vv = np.asarray(slot_values, np.float32)
    rel = np.asarray(reliability, np.float32)
    wd = np.asarray(value_down_w, np.float32)
    wu = np.asarray(value_up_w, np.float32)

    bd = wq.shape[0]
    wst_h = ((kk @ wq) / np.sqrt(np.float32(bd))).T  # [H, N]
    vd_h = vv @ wd.T  # [N, VBD]
    wut_h = 0.5 * wu.T  # [VBD, H]
    logrel_h = np.broadcast_to(
        np.log(np.clip(rel, 1e-10, None)), (P, NSLOT)
    ).astype(np.float32).copy()
    ident_h = np.eye(P, dtype=np.float32)

    shared = {
        "wst": np.ascontiguousarray(wst_h).astype(BF16),
        "vd": np.ascontiguousarray(vd_h).astype(BF16),
        "wut": np.ascontiguousarray(wut_h).astype(BF16),
        "logrel": logrel_h,
        "ident": ident_h.astype(BF16),
    }
    hs2b = hs2.astype(BF16)
    in_maps = []
    for c in range(N_CORES):
        rows = slice(c * TPC, (c + 1) * TPC)
        in_maps.append(
            {
                "hst": np.ascontiguousarray(hs2b[rows].T),
                "paob": np.ascontiguousarray(pao2[rows]).astype(BF16),
                **shared,
            }
        )
    return in_maps


def kernel(**inputs):
    from concourse.bass_utils import run_bass_kernel_spmd

    nc = _get_graph()
    in_maps = _make_in_maps(**inputs)
    res = run_bass_kernel_spmd(nc, in_maps, core_ids=list(range(N_CORES)))
    full = np.concatenate(
        [res.results[c]["out"].astype(np.float32) for c in range(N_CORES)], axis=0
    )
    return full.reshape(B, S, H)


# revision 20
# speedup vs baseline: 1.3182x; 1.3121x over previous
"""Trainium2 Bass kernel for the AGA retrieval-KNN operator (8-core SPMD).

Reference computation (per token):
    q = hs @ Wq.T                        [BD]
    s = q @ K.T / sqrt(BD) + log(rel)    [N]
    top8, idx = top_k(s, 8); attn = softmax(top8); gate = sigmoid(top8[0])
    aux = attn @ V[idx]                  [H]
    aux = gelu(aux @ Wd.T) @ Wu.T        [H]
    out = pao + gate * aux

Kernel strategy:
  - Pure data parallel: 8192 tokens sharded 1024/core; weights replicated.
  - Host-side constant folding: WsT = (K @ Wq).T / sqrt(BD)  [H, N] so the
    score is a single matmul; Vd = V @ Wd.T [N, VBD] so the top-k gather +
    weighted sum + down-projection collapse into one dense [T,N]x[N,VBD]
    matmul against a sparse (top-8 masked) softmax weight vector;
    WuT = 0.5 * Wu.T (0.5 from the tanh-gelu identity gelu(x)=0.5x(1+t)).
  - Top-8 via the DVE max8 instruction; dense masked softmax weights via one
    scalar_tensor_tensor (mask * exp, accumulating the denominator).
  - 1/denom and the sigmoid gate are folded into per-partition scalar
    operands of downstream ops (they commute with the linear ops).
  - gelu computed with the tanh formula (exp/tanh share one ACT table set;
    the Gelu LUT lives in a different set and would thrash table loads).
  - pao is accumulated into the up-projection PSUM with an identity-matmul.
  - bf16 storage/compute everywhere (fp32 PSUM accumulate); fp32 host cast.
"""

import numpy as np
import ml_dtypes

B, S, H = 4, 2048, 2048
NSLOT = 256  # slot pool size == BD == VBD
P = 128
N_CORES = 8
TPC = (B * S) // N_CORES  # tokens per core = 1024
NT = TPC // P  # token tiles per core = 8
KH = H // P  # contraction chunks over H = 16
BF16 = ml_dtypes.bfloat16

_CACHE = {}


def _build_graph():
    import concourse.bacc as bacc
    import concourse.mybir as mybir
    from concourse import tile

    F32 = mybir.dt.float32
    BF = mybir.dt.bfloat16
    AF = mybir.ActivationFunctionType
    ALU = mybir.AluOpType

    nc = bacc.Bacc("TRN2", target_bir_lowering=False, debug=False)

    NP = NSLOT
    hst = nc.dram_tensor("hst", [H, TPC], BF, kind="ExternalInput")
    paob = nc.dram_tensor("paob", [TPC, H], BF, kind="ExternalInput")
    wst = nc.dram_tensor("wst", [H, NP], BF, kind="ExternalInput")
    vd = nc.dram_tensor("vd", [NSLOT, NSLOT], BF, kind="ExternalInput")
    wut = nc.dram_tensor("wut", [NSLOT, H], BF, kind="ExternalInput")
    logrel = nc.dram_tensor("logrel", [P, NP], F32, kind="ExternalInput")
    ident = nc.dram_tensor("ident", [P, P], BF, kind="ExternalInput")
    out = nc.dram_tensor("out", [TPC, H], BF, kind="ExternalOutput")

    with tile.TileContext(nc) as tc:
        with (
            tc.tile_pool(name="const", bufs=1) as cpool,
            tc.tile_pool(name="work", bufs=4) as wpool,
            tc.tile_pool(name="io", bufs=3) as iopool,
            tc.tile_pool(name="ps_s", bufs=2, space="PSUM") as ps_s_pool,
            tc.tile_pool(name="ps_tr", bufs=2, space="PSUM") as ps_tr_pool,
            tc.tile_pool(name="ps_g", bufs=2, space="PSUM") as ps_g_pool,
            tc.tile_pool(name="ps_u", bufs=2, space="PSUM") as ps_u_pool,
        ):
            # ---- resident weights/activations -------------------------------
            # hsT/WsT loaded per 128-row chunk so the k-th score matmul can
            # start as soon as its chunk lands.
            hsT = cpool.tile([P, KH, TPC], BF)
            wst_sb = cpool.tile([P, KH, NP], BF)
            for k in range(KH):
                nc.sync.dma_start(
                    out=wst_sb[:, k, :], in_=wst.ap()[k * P : (k + 1) * P, :]
                )
                nc.sync.dma_start(
                    out=hsT[:, k, :], in_=hst.ap()[k * P : (k + 1) * P, :]
                )
            vd_sb = cpool.tile([P, 2, NSLOT], BF)
            nc.sync.dma_start(
                out=vd_sb[:], in_=vd.ap().rearrange("(k p) n -> p k n", p=P)
            )
            wut_sb = cpool.tile([P, 2, H], BF)
            nc.sync.dma_start(
                out=wut_sb[:], in_=wut.ap().rearrange("(k p) n -> p k n", p=P)
            )
            logrel_sb = cpool.tile([P, NP], F32)
            nc.sync.dma_start(out=logrel_sb[:], in_=logrel.ap())
            ident_sb = cpool.tile([P, P], BF)
            nc.sync.dma_start(out=ident_sb[:], in_=ident.ap())

            state = {}

            def phase_a(t):
                """scores -> top8 -> masked softmax weights w, scales."""
                tok = slice(t * P, (t + 1) * P)
                ps_s = ps_s_pool.tile([P, NP], F32, tag="ps_s", name=f"ps_s{t}")
                for k in range(KH):
                    nc.tensor.matmul(
                        out=ps_s[:],
                        lhsT=hsT[:, k, tok],
                        rhs=wst_sb[:, k, :],
                        start=(k == 0),
                        stop=(k == KH - 1),
                    )
                s_sb = wpool.tile([P, NP], F32, tag="s", name=f"s{t}")
                nc.vector.tensor_tensor(
                    out=s_sb[:], in0=ps_s[:], in1=logrel_sb[:], op=ALU.add
                )
                t8 = wpool.tile([P, 8], F32, tag="t8", name=f"t8_{t}")
                nc.vector.max(out=t8[:], in_=s_sb[:])
                neg_m = wpool.tile([P, 1], F32, tag="negm", name=f"negm{t}")
                nc.vector.tensor_scalar_mul(neg_m[:], t8[:, 0:1], -1.0)
                e_sb = wpool.tile([P, NP], BF, tag="e", name=f"e{t}")
                nc.scalar.activation(e_sb[:], s_sb[:], AF.Exp, bias=neg_m[:])
                pair = wpool.tile([P, 2], F32, tag="pair", name=f"pair{t}")
                # pair[:,1] = exp(-top1)
                nc.scalar.activation(pair[:, 1:2], t8[:, 0:1], AF.Exp, scale=-1.0)
                nc.vector.tensor_scalar_add(pair[:, 1:2], pair[:, 1:2], 1.0)
                w_sb = wpool.tile([P, NSLOT], BF, tag="w", name=f"w{t}")
                # w = (s >= top8) * exp(s - top1); pair[:,0] = sum(w) = denom
                nc.vector.scalar_tensor_tensor(
                    out=w_sb[:],
                    in0=s_sb[:],
                    scalar=t8[:, 7:8],
                    in1=e_sb[:],
                    op0=ALU.is_ge,
                    op1=ALU.mult,
                    accum_out=pair[:, 0:1],
                )
                rec = wpool.tile([P, 2], F32, tag="rec", name=f"rec{t}")
                # rec[:,0] = 1/denom ; rec[:,1] = gate = sigmoid(top1)
                nc.vector.reciprocal(rec[:], pair[:])
                sc = wpool.tile([P, 2], F32, tag="sc", name=f"sc{t}")
                # sc[:,0] = gate/denom ; sc[:,1] = c0/denom (tanh-gelu const)
                nc.vector.tensor_tensor(
                    out=sc[:, 0:1], in0=rec[:, 0:1], in1=rec[:, 1:2], op=ALU.mult
                )
                nc.vector.tensor_scalar_mul(sc[:, 1:2], rec[:, 0:1], 0.7978845608028654)
                pao_t = iopool.tile([P, H], BF, tag="pao", name=f"pao{t}")
                nc.sync.dma_start(out=pao_t[:], in_=paob.ap()[tok, :])
                state[t] = (w_sb, sc, pao_t)

            def phase_b1(t):
                """w -> wT -> g = w @ Vd."""
                w_sb, sc, pao_t = state[t]
                wT = wpool.tile([P, 2, P], BF, tag="wT", name=f"wT{t}")
                for k in range(2):
                    ps_tr = ps_tr_pool.tile([P, P], BF, tag="ptr", name=f"ptw{t}{k}")
                    nc.tensor.transpose(
                        ps_tr[:], w_sb[:, k * P : (k + 1) * P], ident_sb[:]
                    )
                    nc.vector.tensor_copy(wT[:, k, :], ps_tr[:])
                ps_g = ps_g_pool.tile([P, NSLOT], F32, tag="ps_g", name=f"ps_g{t}")
                for k in range(2):
                    nc.tensor.matmul(
                        out=ps_g[:],
                        lhsT=wT[:, k, :],
                        rhs=vd_sb[:, k, :],
                        start=(k == 0),
                        stop=(k == 1),
                    )
                state[t] = (sc, pao_t, ps_g)

            def phase_b2(t):
                """g -> gelu -> u -> + pao -> out."""
                tok = slice(t * P, (t + 1) * P)
                sc, pao_t, ps_g = state.pop(t)
                # gelu(x) ~ 0.5 x (1 + tanh(c0 x)) for tiny x = g/denom (the
                # x^3 term is < 1e-5 relative here; 0.5 folded into WuT).
                # d = (1 + tanh(g * c0/denom)) * g * (gate/denom)
                rr = wpool.tile([P, NSLOT], BF, tag="rr", name=f"rr{t}")
                nc.scalar.activation(rr[:], ps_g[:], AF.Tanh, scale=sc[:, 1:2])
                dp = wpool.tile([P, NSLOT], BF, tag="dp", name=f"dp{t}")
                nc.vector.scalar_tensor_tensor(
                    out=dp[:], in0=rr[:], scalar=1.0, in1=ps_g[:],
                    op0=ALU.add, op1=ALU.mult,
                )
                d_sb = wpool.tile([P, NSLOT], BF, tag="d", name=f"d{t}")
                nc.vector.tensor_scalar_mul(d_sb[:], dp[:], sc[:, 0:1])

                dT = wpool.tile([P, 2, P], BF, tag="dT", name=f"dT{t}")
                for k in range(2):
                    ps_tr = ps_tr_pool.tile([P, P], BF, tag="ptr", name=f"ptd{t}{k}")
                    nc.tensor.transpose(
                        ps_tr[:], d_sb[:, k * P : (k + 1) * P], ident_sb[:]
                    )
                    nc.scalar.activation(dT[:, k, :], ps_tr[:], AF.Copy)

                out_sb = iopool.tile([P, H], BF, tag="out", name=f"o{t}")
                for c in range(4):
                    ps_u = ps_u_pool.tile([P, 512], F32, tag="pu", name=f"pu{t}{c}")
                    cs = slice(c * 512, (c + 1) * 512)
                    nc.tensor.matmul(
                        out=ps_u[:], lhsT=dT[:, 0, :], rhs=wut_sb[:, 0, cs],
                        start=True, stop=False,
                    )
                    with_pe_pao = c < 2
                    nc.tensor.matmul(
                        out=ps_u[:], lhsT=dT[:, 1, :], rhs=wut_sb[:, 1, cs],
                        start=False, stop=not with_pe_pao,
                    )
                    if with_pe_pao:
                        # pao folded in on the PE; evacuate on ACT
                        nc.tensor.matmul(
                            out=ps_u[:], lhsT=ident_sb[:], rhs=pao_t[:, cs],
                            start=False, stop=True,
                        )
                        nc.scalar.activation(out_sb[:, cs], ps_u[:], AF.Copy)
                    else:
                        # pao folded into the DVE evacuation
                        nc.vector.scalar_tensor_tensor(
                            out=out_sb[:, cs], in0=ps_u[:], scalar=0.0,
                            in1=pao_t[:, cs], op0=ALU.add, op1=ALU.add,
                        )
                nc.sync.dma_start(out=out.ap()[tok, :], in_=out_sb[:])

            # 3-stage software pipeline: engines execute their streams
            # in-order, so emit A(t+2) and B1(t+1) before B2(t) -- the PE
            # fills tile t's gelu latency with tile t+2's score matmuls and
            # tile t+1's w-transpose/g-matmul.
            phase_a(0)
            phase_a(1)
            phase_b1(0)
            for t in range(NT):
                if t + 2 < NT:
                    phase_a(t + 2)
                if t + 1 < NT:
                    phase_b1(t + 1)
                phase_b2(t)

    nc.compile()
    return nc


def _get_graph():
    if "nc" not in _CACHE:
        _CACHE["nc"] = _build_graph()
    return _CACHE["nc"]


def _make_in_maps(
    hidden_states,
    primary_attention_output,
    q_proj_w,
    slot_keys,
    slot_values,
    reliability,
    value_down_w,
    value_up_w,
):
    hs2 = np.asarray(hidden_states, np.float32).reshape(-1, H)
    pao2 = np.asarray(primary_attention_output, np.float32).reshape(-1, H)
    wq = np.asarray(q_proj_w, np.float32)
    kk = np.asarray(slot_keys, np.float32)
    vv = np.asarray(slot_values, np.float32)
    rel = np.asarray(reliability, np.float32)
    wd = np.asarray(value_down_w, np.float32)
    wu = np.asarray(value_up_w, np.float32)

    bd = wq.shape[0]
    wst_h = ((kk @ wq) / np.sqrt(np.float32(bd))).T  # [H, N]
    vd_h = vv @ wd.T  # [N, VBD]
    wut_h = 0.5 * wu.T  # [VBD, H]
    logrel_h = np.broadcast_to(
        np.log(np.clip(rel, 1e-10, None)), (P, NSLOT)
    ).astype(np.float32).copy()
    ident_h = np.eye(P, dtype=np.float32)

    shared = {
        "wst": np.ascontiguousarray(wst_h).astype(BF16),
        "vd": np.ascontiguousarray(vd_h).astype(BF16),
        "wut": np.ascontiguousarray(wut_h).astype(BF16),
        "logrel": logrel_h,
        "ident": ident_h.astype(BF16),
    }
    hs2b = hs2.astype(BF16)
    in_maps = []
    for c in range(N_CORES):
        rows = slice(c * TPC, (c + 1) * TPC)
        in_maps.append(
            {
                "hst": np.ascontiguousarray(hs2b[rows].T),
                "paob": np.ascontiguousarray(pao2[rows]).astype(BF16),
                **shared,
            }
        )
    return in_maps


def kernel(**inputs):
    from concourse.bass_utils import run_bass_kernel_spmd

    nc = _get_graph()
    in_maps = _make_in_maps(**inputs)
    res = run_bass_kernel_spmd(nc, in_maps, core_ids=list(range(N_CORES)))
    full = np.concatenate(
        [res.results[c]["out"].astype(np.float32) for c in range(N_CORES)], axis=0
    )
    return full.reshape(B, S, H)


# revision 21
# speedup vs baseline: 1.4853x; 1.1267x over previous
"""Trainium2 Bass kernel for the AGA retrieval-KNN operator (8-core SPMD).

Reference computation (per token):
    q = hs @ Wq.T                        [BD]
    s = q @ K.T / sqrt(BD) + log(rel)    [N]
    top8, idx = top_k(s, 8); attn = softmax(top8); gate = sigmoid(top8[0])
    aux = attn @ V[idx]                  [H]
    aux = gelu(aux @ Wd.T) @ Wu.T        [H]
    out = pao + gate * aux

Kernel strategy:
  - Pure data parallel: 8192 tokens sharded 1024/core; weights replicated.
  - Host-side constant folding: WsT = (K @ Wq).T / sqrt(BD)  [H, N] so the
    score is a single matmul; Vd = V @ Wd.T [N, VBD] so the top-k gather +
    weighted sum + down-projection collapse into one dense [T,N]x[N,VBD]
    matmul against a sparse (top-8 masked) softmax weight vector;
    WuT = 0.5 * Wu.T (0.5 from the tanh-gelu identity gelu(x)=0.5x(1+t)).
  - Top-8 via the DVE max8 instruction; dense masked softmax weights via one
    scalar_tensor_tensor (mask * exp, accumulating the denominator).
  - 1/denom and the sigmoid gate are folded into per-partition scalar
    operands of downstream ops (they commute with the linear ops).
  - gelu computed with the tanh formula (exp/tanh share one ACT table set;
    the Gelu LUT lives in a different set and would thrash table loads).
  - pao is accumulated into the up-projection PSUM with an identity-matmul.
  - bf16 storage/compute everywhere (fp32 PSUM accumulate); fp32 host cast.
"""

import numpy as np
import ml_dtypes

B, S, H = 4, 2048, 2048
NSLOT = 256  # slot pool size == BD == VBD
P = 128
N_CORES = 8
TPC = (B * S) // N_CORES  # tokens per core = 1024
NT = TPC // P  # token tiles per core = 8
KH = H // P  # contraction chunks over H = 16
BF16 = ml_dtypes.bfloat16

_CACHE = {}


def _build_graph():
    import concourse.bacc as bacc
    import concourse.mybir as mybir
    from concourse import tile

    F32 = mybir.dt.float32
    BF = mybir.dt.bfloat16
    AF = mybir.ActivationFunctionType
    ALU = mybir.AluOpType

    nc = bacc.Bacc("TRN2", target_bir_lowering=False, debug=False)

    NP = NSLOT
    hst = nc.dram_tensor("hst", [H, TPC], BF, kind="ExternalInput")
    paob = nc.dram_tensor("paob", [TPC, H], BF, kind="ExternalInput")
    wst = nc.dram_tensor("wst", [H, NP], BF, kind="ExternalInput")
    vd = nc.dram_tensor("vd", [NSLOT, NSLOT], BF, kind="ExternalInput")
    wut = nc.dram_tensor("wut", [NSLOT, H], BF, kind="ExternalInput")
    logrel = nc.dram_tensor("logrel", [P, NP], F32, kind="ExternalInput")
    ident = nc.dram_tensor("ident", [P, P], BF, kind="ExternalInput")
    out = nc.dram_tensor("out", [TPC, H], BF, kind="ExternalOutput")

    with tile.TileContext(nc) as tc:
        with (
            tc.tile_pool(name="const", bufs=1) as cpool,
            tc.tile_pool(name="work", bufs=4) as wpool,
            tc.tile_pool(name="io", bufs=3) as iopool,
            tc.tile_pool(name="ps_s", bufs=2, space="PSUM") as ps_s_pool,
            tc.tile_pool(name="ps_tr", bufs=2, space="PSUM") as ps_tr_pool,
            tc.tile_pool(name="ps_g", bufs=2, space="PSUM") as ps_g_pool,
            tc.tile_pool(name="ps_u", bufs=2, space="PSUM") as ps_u_pool,
        ):
            # ---- resident weights/activations -------------------------------
            # hsT/WsT loaded per 128-row chunk so the k-th score matmul can
            # start as soon as its chunk lands.
            hsT = cpool.tile([P, KH, TPC], BF)
            wst_sb = cpool.tile([P, KH, NP], BF)
            for k in range(KH):
                nc.sync.dma_start(
                    out=wst_sb[:, k, :], in_=wst.ap()[k * P : (k + 1) * P, :]
                )
                nc.sync.dma_start(
                    out=hsT[:, k, :], in_=hst.ap()[k * P : (k + 1) * P, :]
                )
            vd_sb = cpool.tile([P, 2, NSLOT], BF)
            nc.sync.dma_start(
                out=vd_sb[:], in_=vd.ap().rearrange("(k p) n -> p k n", p=P)
            )
            wut_sb = cpool.tile([P, 2, H], BF)
            nc.sync.dma_start(
                out=wut_sb[:], in_=wut.ap().rearrange("(k p) n -> p k n", p=P)
            )
            logrel_sb = cpool.tile([P, NP], F32)
            nc.sync.dma_start(out=logrel_sb[:], in_=logrel.ap())
            ident_sb = cpool.tile([P, P], BF)
            nc.sync.dma_start(out=ident_sb[:], in_=ident.ap())

            state = {}

            def phase_a(t):
                """scores -> top8 -> masked softmax weights w, scales."""
                tok = slice(t * P, (t + 1) * P)
                ps_s = ps_s_pool.tile([P, NP], F32, tag="ps_s", name=f"ps_s{t}")
                for k in range(KH):
                    nc.tensor.matmul(
                        out=ps_s[:],
                        lhsT=hsT[:, k, tok],
                        rhs=wst_sb[:, k, :],
                        start=(k == 0),
                        stop=(k == KH - 1),
                    )
                s_sb = wpool.tile([P, NP], F32, tag="s", name=f"s{t}")
                nc.vector.tensor_tensor(
                    out=s_sb[:], in0=ps_s[:], in1=logrel_sb[:], op=ALU.add
                )
                t8 = wpool.tile([P, 8], F32, tag="t8", name=f"t8_{t}")
                nc.vector.max(out=t8[:], in_=s_sb[:])
                neg_m = wpool.tile([P, 1], F32, tag="negm", name=f"negm{t}")
                nc.vector.tensor_scalar_mul(neg_m[:], t8[:, 0:1], -1.0)
                e_sb = wpool.tile([P, NP], BF, tag="e", name=f"e{t}")
                nc.scalar.activation(e_sb[:], s_sb[:], AF.Exp, bias=neg_m[:])
                pair = wpool.tile([P, 2], F32, tag="pair", name=f"pair{t}")
                # pair[:,1] = exp(-top1)
                nc.scalar.activation(pair[:, 1:2], t8[:, 0:1], AF.Exp, scale=-1.0)
                nc.vector.tensor_scalar_add(pair[:, 1:2], pair[:, 1:2], 1.0)
                w_sb = wpool.tile([P, NSLOT], BF, tag="w", name=f"w{t}")
                # w = (s >= top8) * exp(s - top1); pair[:,0] = sum(w) = denom
                nc.vector.scalar_tensor_tensor(
                    out=w_sb[:],
                    in0=s_sb[:],
                    scalar=t8[:, 7:8],
                    in1=e_sb[:],
                    op0=ALU.is_ge,
                    op1=ALU.mult,
                    accum_out=pair[:, 0:1],
                )
                rec = wpool.tile([P, 2], F32, tag="rec", name=f"rec{t}")
                # rec[:,0] = 1/denom ; rec[:,1] = gate = sigmoid(top1)
                nc.vector.reciprocal(rec[:], pair[:])
                sc = wpool.tile([P, 2], F32, tag="sc", name=f"sc{t}")
                # sc[:,0] = gate/denom ; sc[:,1] = c0/denom (tanh-gelu const)
                nc.vector.tensor_tensor(
                    out=sc[:, 0:1], in0=rec[:, 0:1], in1=rec[:, 1:2], op=ALU.mult
                )
                nc.vector.tensor_scalar_mul(sc[:, 1:2], rec[:, 0:1], 0.7978845608028654)
                state[t] = (w_sb, sc)

            def phase_b1(t):
                """w -> wT -> g = w @ Vd; prefetch pao."""
                w_sb, sc = state[t]
                pao_t = iopool.tile([P, H], BF, tag="pao", name=f"pao{t}")
                nc.sync.dma_start(
                    out=pao_t[:], in_=paob.ap()[t * P : (t + 1) * P, :]
                )
                wT = wpool.tile([P, 2, P], BF, tag="wT", name=f"wT{t}")
                for k in range(2):
                    ps_tr = ps_tr_pool.tile([P, P], BF, tag="ptr", name=f"ptw{t}{k}")
                    nc.tensor.transpose(
                        ps_tr[:], w_sb[:, k * P : (k + 1) * P], ident_sb[:]
                    )
                    nc.vector.tensor_copy(wT[:, k, :], ps_tr[:])
                ps_g = ps_g_pool.tile([P, NSLOT], F32, tag="ps_g", name=f"ps_g{t}")
                for k in range(2):
                    nc.tensor.matmul(
                        out=ps_g[:],
                        lhsT=wT[:, k, :],
                        rhs=vd_sb[:, k, :],
                        start=(k == 0),
                        stop=(k == 1),
                    )
                state[t] = (sc, pao_t, ps_g)

            def phase_b2(t):
                """g -> gelu -> u -> + pao -> out."""
                tok = slice(t * P, (t + 1) * P)
                sc, pao_t, ps_g = state.pop(t)
                # gelu(x) ~ 0.5 x (1 + tanh(c0 x)) for tiny x = g/denom (the
                # x^3 term is < 1e-5 relative here; 0.5 folded into WuT).
                # d = (1 + tanh(g * c0/denom)) * g * (gate/denom)
                rr = wpool.tile([P, NSLOT], BF, tag="rr", name=f"rr{t}")
                nc.scalar.activation(rr[:], ps_g[:], AF.Tanh, scale=sc[:, 1:2])
                dp = wpool.tile([P, NSLOT], BF, tag="dp", name=f"dp{t}")
                nc.vector.scalar_tensor_tensor(
                    out=dp[:], in0=rr[:], scalar=1.0, in1=ps_g[:],
                    op0=ALU.add, op1=ALU.mult,
                )
                d_sb = wpool.tile([P, NSLOT], BF, tag="d", name=f"d{t}")
                nc.vector.tensor_scalar_mul(d_sb[:], dp[:], sc[:, 0:1])

                dT = wpool.tile([P, 2, P], BF, tag="dT", name=f"dT{t}")
                for k in range(2):
                    ps_tr = ps_tr_pool.tile([P, P], BF, tag="ptr", name=f"ptd{t}{k}")
                    nc.tensor.transpose(
                        ps_tr[:], d_sb[:, k * P : (k + 1) * P], ident_sb[:]
                    )
                    nc.scalar.activation(dT[:, k, :], ps_tr[:], AF.Copy)

                out_sb = iopool.tile([P, H], BF, tag="out", name=f"o{t}")
                for c in range(4):
                    ps_u = ps_u_pool.tile([P, 512], F32, tag="pu", name=f"pu{t}{c}")
                    cs = slice(c * 512, (c + 1) * 512)
                    nc.tensor.matmul(
                        out=ps_u[:], lhsT=dT[:, 0, :], rhs=wut_sb[:, 0, cs],
                        start=True, stop=False,
                    )
                    with_pe_pao = c < 2
                    nc.tensor.matmul(
                        out=ps_u[:], lhsT=dT[:, 1, :], rhs=wut_sb[:, 1, cs],
                        start=False, stop=not with_pe_pao,
                    )
                    if with_pe_pao:
                        # pao folded in on the PE; evacuate on ACT
                        nc.tensor.matmul(
                            out=ps_u[:], lhsT=ident_sb[:], rhs=pao_t[:, cs],
                            start=False, stop=True,
                        )
                        nc.scalar.activation(out_sb[:, cs], ps_u[:], AF.Copy)
                    else:
                        # pao folded into the DVE evacuation
                        nc.vector.scalar_tensor_tensor(
                            out=out_sb[:, cs], in0=ps_u[:], scalar=0.0,
                            in1=pao_t[:, cs], op0=ALU.add, op1=ALU.add,
                        )
                nc.sync.dma_start(out=out.ap()[tok, :], in_=out_sb[:])

            # 3-stage software pipeline: engines execute their streams
            # in-order, so emit A(t+2) and B1(t+1) before B2(t) -- the PE
            # fills tile t's gelu latency with tile t+2's score matmuls and
            # tile t+1's w-transpose/g-matmul.
            phase_a(0)
            phase_a(1)
            phase_b1(0)
            for t in range(NT):
                if t + 2 < NT:
                    phase_a(t + 2)
                if t + 1 < NT:
                    phase_b1(t + 1)
                phase_b2(t)

    nc.compile()
    return nc


def _get_graph():
    if "nc" not in _CACHE:
        _CACHE["nc"] = _build_graph()
    return _CACHE["nc"]


def _make_in_maps(
    hidden_states,
    primary_attention_output,
    q_proj_w,
    slot_keys,
    slot_values,
    reliability,
    value_down_w,
    value_up_w,
):
    hs2 = np.asarray(hidden_states, np.float32).reshape(-1, H)
    pao2 = np.asarray(primary_attention_output, np.float32).reshape(-1, H)
    wq = np.asarray(q_proj_w, np.float32)
    kk = np.asarray(slot_keys, np.float32)
    vv = np.asarray(slot_values, np.float32)
    rel = np.asarray(reliability, np.float32)
    wd = np.asarray(value_down_w, np.float32)
    wu = np.asarray(value_up_w, np.float32)

    bd = wq.shape[0]
    wst_h = ((kk @ wq) / np.sqrt(np.float32(bd))).T  # [H, N]
    vd_h = vv @ wd.T  # [N, VBD]
    wut_h = 0.5 * wu.T  # [VBD, H]
    logrel_h = np.broadcast_to(
        np.log(np.clip(rel, 1e-10, None)), (P, NSLOT)
    ).astype(np.float32).copy()
    ident_h = np.eye(P, dtype=np.float32)

    shared = {
        "wst": np.ascontiguousarray(wst_h).astype(BF16),
        "vd": np.ascontiguousarray(vd_h).astype(BF16),
        "wut": np.ascontiguousarray(wut_h).astype(BF16),
        "logrel": logrel_h,
        "ident": ident_h.astype(BF16),
    }
    hs2b = hs2.astype(BF16)
    in_maps = []
    for c in range(N_CORES):
        rows = slice(c * TPC, (c + 1) * TPC)
        in_maps.append(
            {
                "hst": np.ascontiguousarray(hs2b[rows].T),
                "paob": np.ascontiguousarray(pao2[rows]).astype(BF16),
                **shared,
            }
        )
    return in_maps


def kernel(**inputs):
    from concourse.bass_utils import run_bass_kernel_spmd

    nc = _get_graph()
    in_maps = _make_in_maps(**inputs)
    res = run_bass_kernel_spmd(nc, in_maps, core_ids=list(range(N_CORES)))
    full = np.concatenate(
        [res.results[c]["out"].astype(np.float32) for c in range(N_CORES)], axis=0
    )
    return full.reshape(B, S, H)
